# revision 1
# baseline (speedup 1.0000x reference)
"""CSWin block kernel for TRN2, 8-core data-parallel over batch.

Self-contained: hardcodes shapes from the problem spec.
kernel(**inputs) -> (16, 3136, 256) float32.
"""
import os
import numpy as np

import concourse.bass as bass
import concourse.bacc as bacc
import concourse.tile as tile
from concourse import mybir
from concourse.bass_utils import run_bass_kernel_spmd

FP = mybir.dt.float32
F32R = mybir.dt.float32r
FP16 = mybir.dt.float16
AF = mybir.ActivationFunctionType
OP = mybir.AluOpType

B, H, W, C = 16, 56, 56, 256
NCORES = 8
BL = B // NCORES            # images per core
IMG = H * W                 # 3136
NTOK = BL * IMG             # 6272
SCALE = 32 ** -0.5
NT = NTOK // 128            # 49 token tiles
EPS = 1e-5
NWIN = 8                    # windows per image per branch
SLACK = 64

BRGEO = [
    dict(R=56, Cw=7, Cg=9, flatN=504,
         koff=[0, 126, 252, 378], ksz=[126, 126, 126, 126]),
    dict(R=7, Cw=56, Cg=58, flatN=406,
         koff=[0, 102, 204, 306], ksz=[102, 102, 102, 100]),
]

_CACHE = {}


def _lepe_taps(Cg, flatN):
    taps = []
    for t in range(9):
        dy, dx = t // 3 - 1, t % 3 - 1
        s = dy * Cg + dx
        src0, dst0 = max(s, 0), max(-s, 0)
        L = flatN - abs(s)
        if dst0 & 1:
            # f32r psum writes need even offsets; element dst0's source is
            # the zero top-left pad, so skipping it is exact
            dst0 += 1
            src0 += 1
            L -= 1
        L += L & 1
        taps.append((t, dst0, src0, L))
    taps.sort(key=lambda r: (r[1] != 0 or r[2] != 0, r[0]))
    return taps


def _build(zero_fc1_bias):
    nc = bacc.Bacc("TRN2", target_bir_lowering=False, debug=False,
                   num_devices=NCORES)

    x_d = nc.dram_tensor("x", [NTOK, C], FP, kind="ExternalInput").ap()
    wqkv_d = nc.dram_tensor("wqkv", [128, 6, 2, 128], F32R, kind="ExternalInput").ap()
    qkvb_d = nc.dram_tensor("qkvb", [128, 6], FP, kind="ExternalInput").ap()
    diag_d = nc.dram_tensor("diag", [128, 2, 9, 128], F32R, kind="ExternalInput").ap()
    convb_d = nc.dram_tensor("convb", [1, 2, 128], F32R, kind="ExternalInput").ap()
    proj_d = nc.dram_tensor("proj16", [128, 2, 256], FP16, kind="ExternalInput").ap()
    projb_d = nc.dram_tensor("projb16", [1, 256], FP16, kind="ExternalInput").ap()
    fc1_d = nc.dram_tensor("fc1w", [128, 2, 1024], F32R, kind="ExternalInput").ap()
    fc1b_d = nc.dram_tensor("fc1b", [128, 8], FP, kind="ExternalInput").ap()
    fc2_d = nc.dram_tensor("fc2w16", [128, 8, 256], FP16, kind="ExternalInput").ap()
    fc2b_d = nc.dram_tensor("fc2b16", [1, 256], FP16, kind="ExternalInput").ap()
    mask_d = nc.dram_tensor("mask16", [128, 8, 32], FP16, kind="ExternalInput").ap()
    ident_d = nc.dram_tensor("ident", [128, 128], FP, kind="ExternalInput").ap()
    ones_d = nc.dram_tensor("onesr", [1, 512], F32R, kind="ExternalInput").ap()
    ones16_d = nc.dram_tensor("ones16", [1, 128], FP16, kind="ExternalInput").ap()
    out_d = nc.dram_tensor("out", [NTOK, C], FP, kind="ExternalOutput").ap()

    with tile.TileContext(nc) as tc:
        p_w = tc.alloc_tile_pool(name="p_w", bufs=1)
        p_ps = tc.alloc_tile_pool(name="p_ps", bufs=1, space="PSUM")
        p_scr = tc.alloc_tile_pool(name="p_scr", bufs=1)

        # ---- weights/consts into SBUF ----
        def wload(name, shape, dt, src):
            t = p_w.tile(shape, dt, name=name)
            nc.sync.dma_start(t, src)
            return t

        wqkv = wload("wqkv_s", [128, 6, 2, 128], F32R, wqkv_d)
        qkvb = wload("qkvb_s", [128, 6], FP, qkvb_d)
        diag = wload("diag_s", [128, 2, 9, 128], F32R, diag_d)
        convb = wload("convb_s", [1, 2, 128], F32R, convb_d)
        proj16 = wload("proj16_s", [128, 2, 256], FP16, proj_d)
        projb16 = wload("projb16_s", [1, 256], FP16, projb_d)
        fc1w = wload("fc1w_s", [128, 2, 1024], F32R, fc1_d)
        fc1b = wload("fc1b_s", [128, 8], FP, fc1b_d)
        fc2w16 = wload("fc2w16_s", [128, 8, 256], FP16, fc2_d)
        fc2b16 = wload("fc2b16_s", [1, 256], FP16, fc2b_d)
        mask16 = wload("mask16_s", [128, 8, 32], FP16, mask_d)
        ident = wload("ident_s", [128, 128], FP, ident_d)
        eps128 = p_w.tile([128, 1], FP, name="eps128")
        nc.vector.memset(eps128, EPS)
        onesr = wload("onesr_s", [1, 512], F32R, ones_d)
        ones16 = wload("ones16_s", [1, 128], FP16, ones16_d)

        def layernorm(src_d, lnt, phase):
            stats = p_scr.tile([128, NT, 2], FP, name=f"stats{phase}")
            rstd = p_scr.tile([128, NT], FP, name=f"rstd{phase}")
            for t in range(NT):
                xa = p_scr.tile([128, 256], FP, tag="xs", bufs=4,
                                name=f"xa{phase}_{t}")
                nc.sync.dma_start(xa, src_d[128 * t:128 * t + 128, :])
                st6 = p_scr.tile([128, 6], FP, tag="st6", bufs=3,
                                 name=f"st6{phase}_{t}")
                nc.vector.bn_stats(st6, xa)
                nc.vector.bn_aggr(stats[:, t, :], st6)
            lnv = p_scr.tile([128, NT], FP, name=f"lnv{phase}")
            nc.scalar.activation(lnv, stats[:, :, 1], AF.Ln, bias=eps128)
            nc.scalar.activation(rstd, lnv, AF.Exp, scale=-0.5)
            for t0 in range(0, NT, 2):
                n2 = min(2, NT - t0)
                tp = p_ps.tile([128, 2, 2, 128], FP, tag="lepe", bufs=1,
                               name=f"lntp{phase}_{t0}")
                for j in range(n2):
                    t = t0 + j
                    xb = p_scr.tile([128, 256], FP, tag="xs", bufs=4,
                                    name=f"xb{phase}_{t}")
                    nc.sync.dma_start(xb, src_d[128 * t:128 * t + 128, :])
                    ln_t = p_scr.tile([128, 256], FP, tag="lnt", bufs=3,
                                      name=f"lnap{phase}_{t}")
                    nc.vector.tensor_scalar(
                        out=ln_t, in0=xb,
                        scalar1=stats[:, t, 0:1], scalar2=rstd[:, t:t + 1],
                        op0=OP.subtract, op1=OP.mult)
                    for c in range(2):
                        nc.tensor.transpose(tp[:, j, c, :],
                                            ln_t[:, 128 * c:128 * c + 128], ident)
                for c in range(2):
                    if n2 == 2:
                        nc.vector.tensor_copy(
                            lnt[c][:, 128 * t0:128 * t0 + 256], tp[:, :, c, :])
                    else:
                        nc.vector.tensor_copy(
                            lnt[c][:, 128 * t0:128 * t0 + 128], tp[:, 0, c, :])

        # ---- LN1 ----
        p_att = tc.alloc_tile_pool(name="p_att", bufs=1)
        p_lnT = tc.alloc_tile_pool(name="p_lnT", bufs=1)
        p_dram = tc.alloc_tile_pool(name="p_dram", bufs=1, space="DRAM")
        x2t = p_dram.tile([NTOK, 256], FP, name="x2scr")
        ln1t = [p_lnT.tile([128, NTOK + SLACK], F32R, tag="lnT", bufs=2,
                           name=f"ln1t{c}") for c in range(2)]
        for c in range(2):
            nc.gpsimd.memset(ln1t[c][:, NTOK:NTOK + SLACK].bitcast(FP), 0.0)

        layernorm(x_d, ln1t, "a")

        # ---- attention ----
        att = [p_att.tile([128, NTOK + SLACK], FP16, name=f"att{c}")
               for c in range(2)]
        p_aw = tc.alloc_tile_pool(name="p_aw", bufs=1)

        grids = []
        for i in range(2):
            g = {}
            for nm in ("qp", "kp", "vp"):
                t_ = p_aw.tile([128, 512], F32R, name=f"{nm}{i}")
                nc.gpsimd.memset(t_.bitcast(FP), 0.0)
                g[nm] = t_
            grids.append(g)

        taps_c = [_lepe_taps(BRGEO[0]["Cg"], BRGEO[0]["flatN"]),
                  _lepe_taps(BRGEO[1]["Cg"], BRGEO[1]["flatN"])]

        widx = 0
        for img in range(BL):
            ioff = img * IMG
            for br in range(2):
                g = BRGEO[br]
                flatN, koff, ksz = g["flatN"], g["koff"], g["ksz"]
                NQ = 448 if br == 0 else 392
                for wi in range(NWIN):
                    gr = grids[widx % 2]
                    qp, kp, vp = gr["qp"], gr["kp"], gr["vp"]

                    def rhs_win(t):
                        if br == 0:
                            return ln1t[t][:, ioff + 7 * wi: ioff + 7 * wi + IMG] \
                                .rearrange("p (y x) -> p y x", x=56)[:, :, 0:8]
                        return ln1t[t][:, ioff + 392 * wi: ioff + 392 * wi + 392]

                    # qkv per window
                    for qi, dst in ((0, qp), (1, kp), (2, vp)):
                        slot = 2 * qi + br
                        pq = p_ps.tile([128, 512], FP, tag="tr", bufs=2,
                                       name=f"pq{widx}_{qi}")
                        for kc in range(2):
                            nc.tensor.matmul(pq[:, 0:NQ], wqkv[:, slot, kc, :],
                                             rhs_win(kc), start=(kc == 0),
                                             stop=(kc == 1))
                        if br == 0:
                            src = pq[:, 0:448].rearrange(
                                "p (y x) -> p y x", x=8)[:, :, 0:7]
                            dstap = dst[:, 0:504].rearrange(
                                "p (y x) -> p y x", x=9)[:, :, 1:8]
                        else:
                            src = pq[:, 0:392].rearrange("p (y x) -> p y x", x=56)
                            dstap = dst[:, 0:406].rearrange(
                                "p (y x) -> p y x", x=58)[:, :, 1:57]
                        nc.vector.tensor_scalar_add(
                            out=dstap, in0=src, scalar1=qkvb[:, slot:slot + 1])

                    # vT: transpose v_pad chunks -> fp16
                    tv = p_ps.tile([128, 4, 128], FP, tag="tr", bufs=2,
                                   name=f"tv{widx}")
                    for kc in range(4):
                        nc.tensor.transpose(
                            tv[0:ksz[kc], kc, :],
                            vp.bitcast(FP)[:, koff[kc]:koff[kc] + ksz[kc]],
                            ident)
                    vt16 = p_scr.tile([128, 4, 128], FP16, tag="vt16", bufs=2,
                                      name=f"vt16_{widx}")
                    nc.vector.tensor_copy(vt16, tv)

                    # lepe
                    lep = p_ps.tile([128, 512], FP, tag="lepe", bufs=1,
                                    name=f"lep{widx}")
                    for i, (t, dst0, src0, L) in enumerate(taps_c[br]):
                        nc.tensor.matmul(lep[:, dst0:dst0 + L],
                                         diag[:, br, t, :], vp[:, src0:src0 + L],
                                         start=(i == 0), stop=False,
                                         skip_group_check=True)
                    nc.tensor.matmul(lep[:, 0:flatN], convb[:, br, :],
                                     onesr[:, 0:flatN], start=False, stop=True,
                                     skip_group_check=True)

                    # S^T + exp per k-chunk
                    s4 = p_ps.tile([128, 4, 512], FP, tag="s4", bufs=1,
                                   name=f"s4_{widx}")
                    es = []
                    if br == 0:
                        rq = qp[:, 0:504].rearrange(
                            "p (y x) -> p y x", x=9)[:, :, 1:9]
                    else:
                        rq = qp[:, 0:406].rearrange(
                            "p (y x) -> p y x", x=58)[:, :, 1:57]
                    for kc in range(4):
                        kn = ksz[kc]
                        for h in range(4):
                            nc.tensor.matmul(
                                s4[0:kn, h, 0:NQ],
                                kp[32 * h:32 * h + 32, koff[kc]:koff[kc] + kn],
                                rq[32 * h:32 * h + 32],
                                start=True, stop=True, tile_position=(32 * h, 0))
                        e = p_scr.tile([128, 4, 392], FP16, tag="es", bufs=6,
                                       name=f"es{widx}_{kc}")
                        if br == 0:
                            ein = s4[0:kn, :, 0:448].rearrange(
                                "p a (y x) -> p a y x", x=8)[:, :, :, 0:7]
                            eout = e[0:kn].rearrange("p a (y x) -> p a y x", x=7)
                        else:
                            ein = s4[0:kn, :, 0:392]
                            eout = e[0:kn]
                        nc.scalar.activation(eout, ein, AF.Exp, scale=SCALE)
                        es.append(e)

                    # @V (fp16 col-tiled) + D into s4 bank0
                    pat = p_ps.tile([128, 512], FP, tag="attn", bufs=1,
                                    name=f"pat{widx}")
                    for h in range(4):
                        for kc in range(4):
                            kn = ksz[kc]
                            nc.tensor.matmul(
                                pat[32 * h:32 * h + 32, 0:392],
                                vt16[0:kn, kc, 32 * h:32 * h + 32],
                                es[kc][0:kn, h, :],
                                start=(kc == 0), stop=(kc == 3),
                                tile_position=(0, 32 * h))
                    for h in range(4):
                        for kc in range(4):
                            kn = ksz[kc]
                            nc.tensor.matmul(
                                s4[32 * h:32 * h + 32, 0, 0:392],
                                mask16[0:kn, 4 * br + kc, :],
                                es[kc][0:kn, h, :],
                                start=(kc == 0), stop=(kc == 3),
                                tile_position=(0, 32 * h))

                    # normalize + lepe add -> att
                    rec = p_scr.tile([128, 392], FP, tag="rec", bufs=2,
                                     name=f"rec{widx}")
                    nc.vector.reciprocal_approx_fast(out=rec, in_=s4[:, 0, 0:392])
                    if br == 0:
                        oap = att[0][:, ioff + 7 * wi: ioff + 7 * wi + IMG] \
                            .rearrange("p (y x) -> p y x", x=56)[:, :, 0:7]
                        i0 = pat[:, 0:392].rearrange("p (y x) -> p y x", x=7)
                        i1 = rec.rearrange("p (y x) -> p y x", x=7)
                        lint = lep[:, 0:504].rearrange(
                            "p (y x) -> p y x", x=9)[:, :, 1:8]
                    else:
                        oap = att[1][:, ioff + 392 * wi: ioff + 392 * wi + 392] \
                            .rearrange("p (y x) -> p y x", x=56)
                        i0 = pat[:, 0:392].rearrange("p (y x) -> p y x", x=56)
                        i1 = rec.rearrange("p (y x) -> p y x", x=56)
                        lint = lep[:, 0:406].rearrange(
                            "p (y x) -> p y x", x=58)[:, :, 1:57]
                    nc.vector.tensor_tensor(oap, i0, i1, OP.mult)
                    nc.vector.tensor_tensor(oap, oap, lint, OP.add)
                    widx += 1

        p_aw.release()

        # ---- proj + residual -> x2 DRAM scratch ----
        for t in range(NT):
            pp = p_ps.tile([128, 256], FP, tag="tr", bufs=2, name=f"pp{t}")
            for c in range(2):
                nc.tensor.matmul(pp, att[c][:, 128 * t:128 * t + 128],
                                 proj16[:, c, :], start=(c == 0), stop=False)
            nc.tensor.matmul(pp, ones16, projb16, start=False, stop=True)
            xs = p_scr.tile([128, 256], FP, tag="xs", bufs=4, name=f"xs{t}")
            nc.sync.dma_start(xs, x_d[128 * t:128 * t + 128, :])
            x2w = p_scr.tile([128, 256], FP, tag="stg", bufs=3, name=f"x2w{t}")
            nc.vector.tensor_tensor(x2w, pp, xs, OP.add)
            nc.sync.dma_start(x2t[128 * t:128 * t + 128, :], x2w)

        # ---- LN2 ----
        ln2t = [p_lnT.tile([128, NTOK + SLACK], F32R, tag="lnT", bufs=2,
                           name=f"ln2t{c}") for c in range(2)]
        layernorm(x2t, ln2t, "b")

        # ---- MLP ----
        NG = 14
        GT = NTOK // NG  # 448
        for gidx in range(NG):
            f1 = p_ps.tile([128, 4, 512], FP, tag="s4", bufs=1, name=f"f1_{gidx}")
            h1 = p_scr.tile([128, 8, 448], FP16, tag="h1", bufs=2,
                            name=f"h1_{gidx}")
            for quad in range(2):
                for mi in range(4):
                    mc = 4 * quad + mi
                    for kc in range(2):
                        nc.tensor.matmul(
                            f1[:, mi, 0:448],
                            fc1w[:, kc, 128 * mc:128 * mc + 128],
                            ln2t[kc][:, GT * gidx:GT * gidx + GT],
                            start=(kc == 0), stop=(kc == 1))
                if zero_fc1_bias:
                    for mi2 in range(0, 4, 2):
                        nc.scalar.activation(
                            h1[:, 4 * quad + mi2:4 * quad + mi2 + 2, :],
                            f1[:, mi2:mi2 + 2, 0:448], AF.Gelu)
                else:
                    for mi2 in range(4):
                        nc.scalar.activation(
                            h1[:, 4 * quad + mi2, :], f1[:, mi2, 0:448], AF.Gelu,
                            bias=fc1b[:, 4 * quad + mi2:4 * quad + mi2 + 1])
            tok = GT * gidx
            end = tok + GT
            while tok < end:
                p0 = tok % 128
                msz = min(128 - p0, end - tok)
                xt = tok // 128
                f2 = p_ps.tile([128, 256], FP, tag="tr", bufs=2,
                               name=f"f2_{gidx}_{tok}")
                a0 = tok - GT * gidx
                for kc in range(8):
                    nc.tensor.matmul(f2[p0:p0 + msz, :],
                                     h1[:, kc, a0:a0 + msz],
                                     fc2w16[:, kc, :],
                                     start=(kc == 0), stop=False)
                nc.tensor.matmul(f2[p0:p0 + msz, :], ones16[:, 0:msz], fc2b16,
                                 start=False, stop=True)
                xc = p_scr.tile([128, 256], FP, tag="xs", bufs=4,
                                name=f"xc{gidx}_{tok}")
                nc.sync.dma_start(xc[p0:p0 + msz, :], x2t[tok:tok + msz, :])
                stg = p_scr.tile([128, 256], FP, tag="stg", bufs=3,
                                 name=f"stg{gidx}_{tok}")
                nc.vector.tensor_tensor(stg[p0:p0 + msz, :], f2[p0:p0 + msz, :],
                                        xc[p0:p0 + msz, :], OP.add)
                nc.sync.dma_start(out_d[tok:tok + msz, :], stg[p0:p0 + msz, :])
                tok += msz

        p_dram.release()
        p_lnT.release()
        p_att.release()
        p_scr.release()
        p_ps.release()
        p_w.release()

    nc.compile()
    return nc


def _host_prep(inputs):
    f = np.asarray
    x = f(inputs["x"], dtype=np.float32)
    g1 = f(inputs["norm1_g"], dtype=np.float32)
    b1 = f(inputs["norm1_b"], dtype=np.float32)
    qkv_w = f(inputs["qkv_w"], dtype=np.float32)
    qkv_b = f(inputs["qkv_b"], dtype=np.float32)
    W1 = g1[:, None] * qkv_w
    bq = qkv_b + b1 @ qkv_w
    wq = np.stack([W1[:, 0:128], W1[:, 128:256], W1[:, 256:384],
                   W1[:, 384:512], W1[:, 512:640], W1[:, 640:768]], axis=0)
    wqkv = np.ascontiguousarray(
        wq.reshape(6, 2, 128, 128).transpose(2, 0, 1, 3))
    qkvb = np.ascontiguousarray(
        np.stack([bq[0:128], bq[128:256], bq[256:384], bq[384:512],
                  bq[512:640], bq[640:768]], axis=1))
    cw0 = f(inputs["conv_w0"], dtype=np.float32)
    cw1 = f(inputs["conv_w1"], dtype=np.float32)
    diag = np.zeros((128, 2, 9, 128), np.float32)
    idx = np.arange(128)
    for br, cw in ((0, cw0), (1, cw1)):
        for t in range(9):
            diag[idx, br, t, idx] = cw[:, 0, t // 3, t % 3]
    convb = np.ascontiguousarray(
        np.stack([f(inputs["conv_b0"], dtype=np.float32),
                  f(inputs["conv_b1"], dtype=np.float32)])[None])
    proj_w = f(inputs["proj_w"], dtype=np.float32)
    proj16 = np.ascontiguousarray(
        proj_w.reshape(2, 128, 256).transpose(1, 0, 2)).astype(np.float16)
    projb16 = f(inputs["proj_b"], dtype=np.float32)[None].astype(np.float16)
    g2 = f(inputs["norm2_g"], dtype=np.float32)
    b2 = f(inputs["norm2_b"], dtype=np.float32)
    fc1_w = f(inputs["fc1_w"], dtype=np.float32)
    W2 = g2[:, None] * fc1_w
    fb1 = f(inputs["fc1_b"], dtype=np.float32) + b2 @ fc1_w
    fc1w = np.ascontiguousarray(W2.reshape(2, 128, 1024).transpose(1, 0, 2))
    fc1b = np.ascontiguousarray(fb1.reshape(8, 128).T)
    fc2_w = f(inputs["fc2_w"], dtype=np.float32)
    fc2w16 = np.ascontiguousarray(
        fc2_w.reshape(8, 128, 256).transpose(1, 0, 2)).astype(np.float16)
    fc2b16 = f(inputs["fc2_b"], dtype=np.float32)[None].astype(np.float16)
    mask = np.zeros((128, 8, 32), np.float16)
    for kc in range(4):
        for br in range(2):
            gg = BRGEO[br]
            ko, kn = gg["koff"][kc], gg["ksz"][kc]
            jj = np.arange(kn)
            valid = (((ko + jj) % gg["Cg"]) != 0) & \
                    (((ko + jj) % gg["Cg"]) != gg["Cg"] - 1)
            mask[0:kn, 4 * br + kc, :] = valid[:, None].astype(np.float16)
    ident = np.eye(128, dtype=np.float32)
    onesr = np.ones((1, 512), np.float32)
    ones16 = np.ones((1, 128), np.float16)

    shared = dict(wqkv=wqkv, qkvb=qkvb, diag=diag, convb=convb,
                  proj16=proj16, projb16=projb16, fc1w=fc1w, fc1b=fc1b,
                  fc2w16=fc2w16, fc2b16=fc2b16, mask16=mask, ident=ident,
                  onesr=onesr, ones16=ones16)
    zero_fc1_bias = not np.any(fb1)
    xs = x.reshape(B, IMG, C)
    in_maps = []
    for core in range(NCORES):
        m = dict(shared)
        m["x"] = np.ascontiguousarray(
            xs[BL * core:BL * core + BL].reshape(NTOK, C))
        in_maps.append(m)
    return in_maps, zero_fc1_bias


def kernel(**inputs):
    in_maps, zero_fc1_bias = _host_prep(inputs)
    key = ("k", zero_fc1_bias)
    if key not in _CACHE:
        _CACHE[key] = _build(zero_fc1_bias)
    nc = _CACHE[key]
    trace = os.environ.get("CSWIN_TRACE", "0") == "1"
    res = run_bass_kernel_spmd(nc, in_maps, core_ids=list(range(NCORES)),
                               trace=trace)
    if trace:
        print("HW exec time:", res.exec_time_ns, "ns")
        kernel.last_results = res
    out = np.concatenate([np.asarray(r["out"]).reshape(BL, IMG, C)
                          for r in res.results], axis=0)
    return out.astype(np.float32)



# revision 11
# speedup vs baseline: 1.1269x; 1.1269x over previous
"""CSWin block kernel for TRN2, 8-core data-parallel over batch.

fp8 (e4m3) DoubleRow matmuls for qkv / lepe / attn@V / softmax-denominator /
proj / MLP; S=QK^T stays fp16. Self-contained: hardcodes shapes.
kernel(**inputs) -> (16, 3136, 256) float32.
"""
import os
import numpy as np
import ml_dtypes

import concourse.bass as bass
import concourse.bacc as bacc
import concourse.tile as tile
from concourse import mybir
from concourse.bass_utils import run_bass_kernel_spmd

FP = mybir.dt.float32
FP16 = mybir.dt.float16
FP8 = mybir.dt.float8e4
AF = mybir.ActivationFunctionType
OP = mybir.AluOpType
PM = mybir.MatmulPerfMode

B, H, W, C = 16, 56, 56, 256
NCORES = 8
BL = B // NCORES            # images per core
IMG = H * W                 # 3136
NTOK = BL * IMG             # 6272
SCALE = 32 ** -0.5
NT = NTOK // 128            # 49 token tiles
EPS = 1e-5
NWIN = 8                    # windows per image per branch
SLACK = 64
MARG = 64                   # zero margin each side of vp8 for shifted lepe taps

BRGEO = [
    dict(R=56, Cw=7, Cg=9, flatN=504,
         koff=[0, 126, 252, 378], ksz=[126, 126, 126, 126]),
    dict(R=7, Cw=56, Cg=58, flatN=406,
         koff=[0, 102, 204, 305], ksz=[102, 102, 101, 101]),
]

_CACHE = {}


def _tap_shifts(Cg):
    return [(t // 3 - 1) * Cg + (t % 3 - 1) for t in range(9)]


def _pair_ap(tl, off, delta, n):
    """[128, 2, n] view of flat tile tl: plane s at column off + s*delta."""
    a = tl[:, off:off + n].unsqueeze(1)
    lst = a.ap
    lst[1] = [delta, 2]
    a.ap = lst
    return a


def _build(zero_fc1_bias):
    nc = bacc.Bacc("TRN2", target_bir_lowering=False, debug=False,
                   num_devices=NCORES)

    x_d = nc.dram_tensor("x", [NTOK, C], FP, kind="ExternalInput").ap()
    wqkv_d = nc.dram_tensor("wqkv8", [128, 6, 2, 128], FP8, kind="ExternalInput").ap()
    qkvb_d = nc.dram_tensor("qkvb", [128, 6], FP, kind="ExternalInput").ap()
    diag_d = nc.dram_tensor("diag8", [128, 2, 5, 2, 128], FP8, kind="ExternalInput").ap()
    convb_d = nc.dram_tensor("convb8", [1, 2, 128], FP8, kind="ExternalInput").ap()
    proj_d = nc.dram_tensor("proj8", [128, 2, 256], FP8, kind="ExternalInput").ap()
    projb_d = nc.dram_tensor("projb16", [1, 256], FP16, kind="ExternalInput").ap()
    fc1_d = nc.dram_tensor("fc1w8", [128, 2, 1024], FP8, kind="ExternalInput").ap()
    fc1b_d = nc.dram_tensor("fc1b", [128, 8], FP, kind="ExternalInput").ap()
    fc2_d = nc.dram_tensor("fc2w8", [128, 8, 256], FP8, kind="ExternalInput").ap()
    fc2b_d = nc.dram_tensor("fc2b16", [1, 256], FP16, kind="ExternalInput").ap()
    mask_d = nc.dram_tensor("mask8", [128, 2, 4, 2, 2, 128], FP8, kind="ExternalInput").ap()
    ident_d = nc.dram_tensor("ident16", [128, 128], FP16, kind="ExternalInput").ap()
    ones8_d = nc.dram_tensor("ones8", [1, 512], FP8, kind="ExternalInput").ap()
    ones16_d = nc.dram_tensor("ones16", [1, 128], FP16, kind="ExternalInput").ap()
    out_d = nc.dram_tensor("out", [NTOK, C], FP, kind="ExternalOutput").ap()

    with tile.TileContext(nc) as tc:
        p_w = tc.alloc_tile_pool(name="p_w", bufs=1)
        p_ps = tc.alloc_tile_pool(name="p_ps", bufs=1, space="PSUM")
        p_scr = tc.alloc_tile_pool(name="p_scr", bufs=1)

        def wload(name, shape, dt, src):
            t = p_w.tile(shape, dt, name=name)
            nc.sync.dma_start(t, src)
            return t

        wqkv8 = wload("wqkv8_s", [128, 6, 2, 128], FP8, wqkv_d)
        qkvb = wload("qkvb_s", [128, 6], FP, qkvb_d)
        diag8 = wload("diag8_s", [128, 2, 5, 2, 128], FP8, diag_d)
        convb8 = wload("convb8_s", [1, 2, 128], FP8, convb_d)
        proj8 = wload("proj8_s", [128, 2, 256], FP8, proj_d)
        projb16 = wload("projb16_s", [1, 256], FP16, projb_d)
        fc1w8 = wload("fc1w8_s", [128, 2, 1024], FP8, fc1_d)
        fc1b = wload("fc1b_s", [128, 8], FP, fc1b_d)
        fc2w8 = wload("fc2w8_s", [128, 8, 256], FP8, fc2_d)
        fc2b16 = wload("fc2b16_s", [1, 256], FP16, fc2b_d)
        mask8 = wload("mask8_s", [128, 2, 4, 2, 2, 128], FP8, mask_d)
        ident16 = wload("ident16_s", [128, 128], FP16, ident_d)
        eps128 = p_w.tile([128, 1], FP, name="eps128")
        nc.vector.memset(eps128, EPS)
        ones8 = wload("ones8_s", [1, 512], FP8, ones8_d)
        ones16 = wload("ones16_s", [1, 128], FP16, ones16_d)

        def layernorm(src_d, ln8, phase):
            stats = p_scr.tile([128, NT, 2], FP, name=f"stats{phase}")
            rstd = p_scr.tile([128, NT], FP, name=f"rstd{phase}")
            for t in range(NT):
                xa = p_scr.tile([128, 256], FP, tag="xs", bufs=4,
                                name=f"xa{phase}_{t}")
                nc.sync.dma_start(xa, src_d[128 * t:128 * t + 128, :])
                st6 = p_scr.tile([128, 6], FP, tag="st6", bufs=3,
                                 name=f"st6{phase}_{t}")
                nc.vector.bn_stats(st6, xa)
                nc.vector.bn_aggr(stats[:, t, :], st6)
            lnv = p_scr.tile([128, NT], FP, name=f"lnv{phase}")
            nc.scalar.activation(lnv, stats[:, :, 1], AF.Ln, bias=eps128)
            nc.scalar.activation(rstd, lnv, AF.Exp, scale=-0.5)
            for t0 in range(0, NT, 2):
                n2 = min(2, NT - t0)
                tp = p_ps.tile([128, 2, 2, 128], FP16, tag="lepe", bufs=1,
                               name=f"lntp{phase}_{t0}")
                for j in range(n2):
                    t = t0 + j
                    xb = p_scr.tile([128, 256], FP, tag="xs", bufs=4,
                                    name=f"xb{phase}_{t}")
                    nc.sync.dma_start(xb, src_d[128 * t:128 * t + 128, :])
                    ln_t = p_scr.tile([128, 256], FP16, tag="lnt", bufs=3,
                                      name=f"lnap{phase}_{t}")
                    nc.vector.tensor_scalar(
                        out=ln_t, in0=xb,
                        scalar1=stats[:, t, 0:1], scalar2=rstd[:, t:t + 1],
                        op0=OP.subtract, op1=OP.mult)
                    for c in range(2):
                        nc.tensor.transpose(tp[:, j, c, :],
                                            ln_t[:, 128 * c:128 * c + 128],
                                            ident16)
                for c in range(2):
                    if n2 == 2:
                        nc.vector.tensor_copy(
                            ln8[:, c, 128 * t0:128 * t0 + 256], tp[:, :, c, :])
                    else:
                        nc.vector.tensor_copy(
                            ln8[:, c, 128 * t0:128 * t0 + 128], tp[:, 0, c, :])

        # ---- LN1 ----
        p_att = tc.alloc_tile_pool(name="p_att", bufs=1)
        p_lnT = tc.alloc_tile_pool(name="p_lnT", bufs=1)
        p_dram = tc.alloc_tile_pool(name="p_dram", bufs=1, space="DRAM")
        x2t = p_dram.tile([NTOK, 256], FP, name="x2scr")
        ln8 = p_lnT.tile([128, 2, NTOK + SLACK], FP8, tag="lnT", bufs=2,
                         name="ln1t8")
        nc.gpsimd.memset(ln8[:, :, NTOK:NTOK + SLACK].bitcast(FP), 0.0)

        layernorm(x_d, ln8, "a")

        # ---- attention ----
        att8 = p_att.tile([128, 2, NTOK + SLACK], FP8, name="att8")
        p_aw = tc.alloc_tile_pool(name="p_aw", bufs=1)

        grids = []
        for i in range(2):
            g = {}
            for nm in ("qp", "kp"):
                t_ = p_aw.tile([128, 512], FP16, name=f"{nm}{i}")
                nc.gpsimd.memset(t_.bitcast(FP), 0.0)
                g[nm] = t_
            v16 = p_aw.tile([128, 512], FP16, name=f"v16_{i}")
            nc.gpsimd.memset(v16.bitcast(FP), 0.0)
            g["v16"] = v16
            v8 = p_aw.tile([128, 2 * MARG + 512], FP8, name=f"v8_{i}")
            nc.gpsimd.memset(v8.bitcast(FP), 0.0)
            g["v8"] = v8
            # block-diagonal v^T for head-pair DoubleRow @V:
            # [kc(4), hpair(2), s(2), 128]; off-block cells stay zero
            vtb = p_aw.tile([128, 4, 2, 2, 128], FP8, name=f"vtb{i}")
            nc.gpsimd.memset(vtb.bitcast(FP), 0.0)
            g["vtb"] = vtb
            grids.append(g)

        shifts = [_tap_shifts(BRGEO[0]["Cg"]), _tap_shifts(BRGEO[1]["Cg"])]

        widx = 0
        for img in range(BL):
            ioff = img * IMG
            for br in range(2):
                g = BRGEO[br]
                flatN, koff, ksz = g["flatN"], g["koff"], g["ksz"]
                Cg = g["Cg"]
                NQ = 448 if br == 0 else 392
                for wi in range(NWIN):
                    gr = grids[widx % 2]
                    qp, kp, v16, v8 = gr["qp"], gr["kp"], gr["v16"], gr["v8"]
                    vtb = gr["vtb"]

                    def rhs_win():
                        if br == 0:
                            return ln8[:, :, ioff + 7 * wi: ioff + 7 * wi + IMG] \
                                .rearrange("p k (y x) -> p k y x", x=56)[:, :, :, 0:8]
                        return ln8[:, :, ioff + 392 * wi: ioff + 392 * wi + 392]

                    # qkv per window (fp8 DoubleRow over the 2 k-chunks)
                    for qi, dst in ((0, qp), (1, kp), (2, v16)):
                        slot = 2 * qi + br
                        pq = p_ps.tile([128, 512], FP, tag="tr", bufs=2,
                                       name=f"pq{widx}_{qi}")
                        nc.tensor.matmul(pq[:, 0:NQ], wqkv8[:, slot, :, :],
                                         rhs_win(), start=True, stop=True,
                                         perf_mode=PM.DoubleRow)
                        if br == 0:
                            src = pq[:, 0:448].rearrange(
                                "p (y x) -> p y x", x=8)[:, :, 0:7]
                            dstap = dst[:, 0:504].rearrange(
                                "p (y x) -> p y x", x=9)[:, :, 1:8]
                        else:
                            src = pq[:, 0:392].rearrange("p (y x) -> p y x", x=56)
                            dstap = dst[:, 0:406].rearrange(
                                "p (y x) -> p y x", x=58)[:, :, 1:57]
                        nc.vector.tensor_scalar_add(
                            out=dstap, in0=src, scalar1=qkvb[:, slot:slot + 1])

                    # zero v guard columns for this branch geometry, then fp8 copy
                    gv = v16[:, 0:flatN].rearrange(
                        "p (y x) -> p y x", x=Cg)[:, :, 0:Cg:Cg - 1]
                    nc.vector.memset(gv, 0.0)
                    nc.vector.tensor_copy(v8[:, MARG:MARG + flatN],
                                          v16[:, 0:flatN])

                    # vT: transpose v16 chunks (fp16), scatter into block-diag
                    # fp8 layout for head-pair DoubleRow
                    tv = p_ps.tile([128, 4, 128], FP16, tag="tr", bufs=2,
                                   name=f"tv{widx}")
                    for kc in range(4):
                        nc.tensor.transpose(
                            tv[0:ksz[kc], kc, :],
                            v16[:, koff[kc]:koff[kc] + ksz[kc]],
                            ident16)
                    for kc in range(4):
                        kn = ksz[kc]
                        for j2 in range(2):
                            src = tv[0:kn, kc, 64 * j2:64 * j2 + 64].rearrange(
                                "p (s c) -> p s c", c=32)
                            # dst cells (s, c): plane s, cols 32*(2*j2+s)+c
                            # -> flat 512*kc + 320*j2 + 160*s + c
                            dst = vtb[0:kn, kc, j2, 0,
                                      64 * j2:64 * j2 + 32].unsqueeze(1)
                            lst = dst.ap
                            lst[1] = [160, 2]
                            dst.ap = lst
                            nc.vector.tensor_copy(dst, src)

                    # lepe: 4 DoubleRow tap-pairs + 1 single tap + bias
                    lep = p_ps.tile([128, 512], FP, tag="lepe", bufs=1,
                                    name=f"lep{widx}")
                    sh = shifts[br]
                    for j in range(4):
                        s0, s1 = sh[2 * j], sh[2 * j + 1]
                        rhs = _pair_ap(v8, MARG + s0, s1 - s0, flatN)
                        nc.tensor.matmul(lep[:, 0:flatN],
                                         diag8[:, br, j, :, :], rhs,
                                         start=(j == 0), stop=False,
                                         perf_mode=PM.DoubleRow,
                                         skip_group_check=True)
                    nc.tensor.matmul(lep[:, 0:flatN], diag8[:, br, 4, 0, :],
                                     v8[:, MARG + sh[8]:MARG + sh[8] + flatN],
                                     start=False, stop=False,
                                     skip_group_check=True)
                    nc.tensor.matmul(lep[:, 0:flatN], convb8[:, br, :],
                                     ones8[:, 0:flatN], start=False, stop=True,
                                     skip_group_check=True)

                    # S^T (fp16) + exp per k-chunk -> fp8
                    s4 = p_ps.tile([128, 4, 512], FP, tag="s4", bufs=1,
                                   name=f"s4_{widx}")
                    es = []
                    if br == 0:
                        rq = qp[:, 0:504].rearrange(
                            "p (y x) -> p y x", x=9)[:, :, 1:9]
                    else:
                        rq = qp[:, 0:406].rearrange(
                            "p (y x) -> p y x", x=58)[:, :, 1:57]
                    for kc in range(4):
                        kn = ksz[kc]
                        for h in range(4):
                            nc.tensor.matmul(
                                s4[0:kn, h, 0:NQ],
                                kp[32 * h:32 * h + 32, koff[kc]:koff[kc] + kn],
                                rq[32 * h:32 * h + 32],
                                start=True, stop=True, tile_position=(32 * h, 0))
                        e = p_scr.tile([128, 4, 392], FP8, tag="es", bufs=6,
                                       name=f"es{widx}_{kc}")
                        if br == 0:
                            ein = s4[0:kn, :, 0:448].rearrange(
                                "p a (y x) -> p a y x", x=8)[:, :, :, 0:7]
                            eout = e[0:kn].rearrange("p a (y x) -> p a y x", x=7)
                        else:
                            ein = s4[0:kn, :, 0:392]
                            eout = e[0:kn]
                        nc.scalar.activation(eout, ein, AF.Exp, scale=SCALE)
                        es.append(e)

                    # @V + D: head-pair DoubleRow, block-diag lhsT, accumulate
                    # over the 4 k-chunks; dst always at partition 0
                    pat = p_ps.tile([128, 512], FP, tag="attn", bufs=1,
                                    name=f"pat{widx}")
                    for kc in range(4):
                        kn = ksz[kc]
                        for j2 in range(2):
                            nc.tensor.matmul(
                                pat[:, 0:392],
                                vtb[0:kn, kc, j2, :, :],
                                es[kc][0:kn, 2 * j2:2 * j2 + 2, :],
                                start=(kc == 0 and j2 == 0),
                                stop=(kc == 3 and j2 == 1),
                                perf_mode=PM.DoubleRow)
                    for kc in range(4):
                        kn = ksz[kc]
                        for j2 in range(2):
                            nc.tensor.matmul(
                                s4[:, 0, 0:392],
                                mask8[0:kn, br, kc, j2, :, :],
                                es[kc][0:kn, 2 * j2:2 * j2 + 2, :],
                                start=(kc == 0 and j2 == 0),
                                stop=(kc == 3 and j2 == 1),
                                perf_mode=PM.DoubleRow)

                    # normalize + lepe add -> att8
                    rec = p_scr.tile([128, 392], FP, tag="rec", bufs=2,
                                     name=f"rec{widx}")
                    nc.vector.reciprocal_approx_fast(out=rec, in_=s4[:, 0, 0:392])
                    tmp = p_scr.tile([128, 392], FP, tag="atmp", bufs=2,
                                     name=f"atmp{widx}")
                    nc.vector.tensor_tensor(tmp, pat[:, 0:392], rec, OP.mult)
                    if br == 0:
                        oap = att8[:, 0, ioff + 7 * wi: ioff + 7 * wi + IMG] \
                            .rearrange("p (y x) -> p y x", x=56)[:, :, 0:7]
                        i0 = tmp.rearrange("p (y x) -> p y x", x=7)
                        lint = lep[:, 0:504].rearrange(
                            "p (y x) -> p y x", x=9)[:, :, 1:8]
                    else:
                        oap = att8[:, 1, ioff + 392 * wi: ioff + 392 * wi + 392] \
                            .rearrange("p (y x) -> p y x", x=56)
                        i0 = tmp.rearrange("p (y x) -> p y x", x=56)
                        lint = lep[:, 0:406].rearrange(
                            "p (y x) -> p y x", x=58)[:, :, 1:57]
                    nc.vector.tensor_tensor(oap, i0, lint, OP.add)
                    widx += 1

        p_aw.release()

        # ---- proj (fp8 DoubleRow) + residual -> x2 DRAM scratch ----
        for t in range(NT):
            pp = p_ps.tile([128, 256], FP, tag="tr", bufs=2, name=f"pp{t}")
            nc.tensor.matmul(pp, att8[:, :, 128 * t:128 * t + 128],
                             proj8, start=True, stop=False,
                             perf_mode=PM.DoubleRow)
            nc.tensor.matmul(pp, ones16, projb16, start=False, stop=True)
            xs = p_scr.tile([128, 256], FP, tag="xs", bufs=4, name=f"xs{t}")
            nc.sync.dma_start(xs, x_d[128 * t:128 * t + 128, :])
            x2w = p_scr.tile([128, 256], FP, tag="stg", bufs=3, name=f"x2w{t}")
            nc.vector.tensor_tensor(x2w, pp, xs, OP.add)
            nc.sync.dma_start(x2t[128 * t:128 * t + 128, :], x2w)

        # ---- LN2 ----
        ln28 = p_lnT.tile([128, 2, NTOK + SLACK], FP8, tag="lnT", bufs=2,
                          name="ln2t8")
        layernorm(x2t, ln28, "b")

        # ---- MLP (fp8 DoubleRow) ----
        NG = 14
        GT = NTOK // NG  # 448
        for gidx in range(NG):
            f1 = p_ps.tile([128, 4, 512], FP, tag="s4", bufs=1, name=f"f1_{gidx}")
            h8 = p_scr.tile([128, 8, 448], FP8, tag="h1", bufs=2,
                            name=f"h1_{gidx}")
            for quad in range(2):
                for mi in range(4):
                    mc = 4 * quad + mi
                    nc.tensor.matmul(
                        f1[:, mi, 0:448],
                        fc1w8[:, :, 128 * mc:128 * mc + 128],
                        ln28[:, :, GT * gidx:GT * gidx + GT],
                        start=True, stop=True, perf_mode=PM.DoubleRow)
                if zero_fc1_bias:
                    for mi2 in range(0, 4, 2):
                        nc.scalar.activation(
                            h8[:, 4 * quad + mi2:4 * quad + mi2 + 2, :],
                            f1[:, mi2:mi2 + 2, 0:448], AF.Gelu)
                else:
                    for mi2 in range(4):
                        nc.scalar.activation(
                            h8[:, 4 * quad + mi2, :], f1[:, mi2, 0:448], AF.Gelu,
                            bias=fc1b[:, 4 * quad + mi2:4 * quad + mi2 + 1])
            tok = GT * gidx
            end = tok + GT
            while tok < end:
                msz = min(128, end - tok)
                f2 = p_ps.tile([128, 256], FP, tag="tr", bufs=2,
                               name=f"f2_{gidx}_{tok}")
                a0 = tok - GT * gidx
                for j in range(4):
                    nc.tensor.matmul(f2[0:msz, :],
                                     h8[:, 2 * j:2 * j + 2, a0:a0 + msz],
                                     fc2w8[:, 2 * j:2 * j + 2, :],
                                     start=(j == 0), stop=False,
                                     perf_mode=PM.DoubleRow)
                nc.tensor.matmul(f2[0:msz, :], ones16[:, 0:msz], fc2b16,
                                 start=False, stop=True)
                xc = p_scr.tile([128, 256], FP, tag="xs", bufs=4,
                                name=f"xc{gidx}_{tok}")
                nc.sync.dma_start(xc[0:msz, :], x2t[tok:tok + msz, :])
                stg = p_scr.tile([128, 256], FP, tag="stg", bufs=3,
                                 name=f"stg{gidx}_{tok}")
                nc.vector.tensor_tensor(stg[0:msz, :], f2[0:msz, :],
                                        xc[0:msz, :], OP.add)
                nc.sync.dma_start(out_d[tok:tok + msz, :], stg[0:msz, :])
                tok += msz

        p_dram.release()
        p_lnT.release()
        p_att.release()
        p_scr.release()
        p_ps.release()
        p_w.release()

    nc.compile()
    return nc


def _host_prep(inputs):
    f = np.asarray
    f8 = ml_dtypes.float8_e4m3
    x = f(inputs["x"], dtype=np.float32)
    g1 = f(inputs["norm1_g"], dtype=np.float32)
    b1 = f(inputs["norm1_b"], dtype=np.float32)
    qkv_w = f(inputs["qkv_w"], dtype=np.float32)
    qkv_b = f(inputs["qkv_b"], dtype=np.float32)
    W1 = g1[:, None] * qkv_w
    bq = qkv_b + b1 @ qkv_w
    wq = np.stack([W1[:, 0:128], W1[:, 128:256], W1[:, 256:384],
                   W1[:, 384:512], W1[:, 512:640], W1[:, 640:768]], axis=0)
    wqkv8 = np.ascontiguousarray(
        wq.reshape(6, 2, 128, 128).transpose(2, 0, 1, 3)).astype(f8)
    qkvb = np.ascontiguousarray(
        np.stack([bq[0:128], bq[128:256], bq[256:384], bq[384:512],
                  bq[512:640], bq[640:768]], axis=1))
    cw0 = f(inputs["conv_w0"], dtype=np.float32)
    cw1 = f(inputs["conv_w1"], dtype=np.float32)
    diag = np.zeros((128, 2, 5, 2, 128), np.float32)
    idx = np.arange(128)
    for br, cw in ((0, cw0), (1, cw1)):
        for t in range(9):
            j, s = t // 2, t % 2
            diag[idx, br, j, s, idx] = cw[:, 0, t // 3, t % 3]
    diag8 = diag.astype(f8)
    convb8 = np.ascontiguousarray(
        np.stack([f(inputs["conv_b0"], dtype=np.float32),
                  f(inputs["conv_b1"], dtype=np.float32)])[None]).astype(f8)
    proj_w = f(inputs["proj_w"], dtype=np.float32)
    proj8 = np.ascontiguousarray(
        proj_w.reshape(2, 128, 256).transpose(1, 0, 2)).astype(f8)
    projb16 = f(inputs["proj_b"], dtype=np.float32)[None].astype(np.float16)
    g2 = f(inputs["norm2_g"], dtype=np.float32)
    b2 = f(inputs["norm2_b"], dtype=np.float32)
    fc1_w = f(inputs["fc1_w"], dtype=np.float32)
    W2 = g2[:, None] * fc1_w
    fb1 = f(inputs["fc1_b"], dtype=np.float32) + b2 @ fc1_w
    fc1w8 = np.ascontiguousarray(
        W2.reshape(2, 128, 1024).transpose(1, 0, 2)).astype(f8)
    fc1b = np.ascontiguousarray(fb1.reshape(8, 128).T)
    fc2_w = f(inputs["fc2_w"], dtype=np.float32)
    fc2w8 = np.ascontiguousarray(
        fc2_w.reshape(8, 128, 256).transpose(1, 0, 2)).astype(f8)
    fc2b16 = f(inputs["fc2_b"], dtype=np.float32)[None].astype(np.float16)
    mask = np.zeros((128, 2, 4, 2, 2, 128), np.float32)
    for br in range(2):
        gg = BRGEO[br]
        for kc in range(4):
            ko, kn = gg["koff"][kc], gg["ksz"][kc]
            jj = np.arange(kn)
            valid = (((ko + jj) % gg["Cg"]) != 0) & \
                    (((ko + jj) % gg["Cg"]) != gg["Cg"] - 1)
            for j2 in range(2):
                for s in range(2):
                    c0 = 64 * j2 + 32 * s
                    mask[0:kn, br, kc, j2, s, c0:c0 + 32] = valid[:, None]
    mask8 = mask.astype(f8)
    ident16 = np.eye(128, dtype=np.float16)
    ones8 = np.ones((1, 512), np.float32).astype(f8)
    ones16 = np.ones((1, 128), np.float16)

    shared = dict(wqkv8=wqkv8, qkvb=qkvb, diag8=diag8, convb8=convb8,
                  proj8=proj8, projb16=projb16, fc1w8=fc1w8, fc1b=fc1b,
                  fc2w8=fc2w8, fc2b16=fc2b16, mask8=mask8, ident16=ident16,
                  ones8=ones8, ones16=ones16)
    zero_fc1_bias = not np.any(fb1)
    xs = x.reshape(B, IMG, C)
    in_maps = []
    for core in range(NCORES):
        m = dict(shared)
        m["x"] = np.ascontiguousarray(
            xs[BL * core:BL * core + BL].reshape(NTOK, C))
        in_maps.append(m)
    return in_maps, zero_fc1_bias


def kernel(**inputs):
    in_maps, zero_fc1_bias = _host_prep(inputs)
    key = ("k8", zero_fc1_bias)
    if key not in _CACHE:
        _CACHE[key] = _build(zero_fc1_bias)
    nc = _CACHE[key]
    trace = os.environ.get("CSWIN_TRACE", "0") == "1"
    res = run_bass_kernel_spmd(nc, in_maps, core_ids=list(range(NCORES)),
                               trace=trace)
    if trace:
        print("HW exec time:", res.exec_time_ns, "ns")
        kernel.last_results = res
    out = np.concatenate([np.asarray(r["out"]).reshape(BL, IMG, C)
                          for r in res.results], axis=0)
    return out.astype(np.float32)
